# revision 10
# baseline (speedup 1.0000x reference)
"""Binary-AlexNet forward pass on 8 Trainium2 NeuronCores (Bass/Tile).

Strategy
--------
Data parallel over the batch: each of the 8 cores runs 16 images through the
conv stack; the fully-connected layers are model-parallel (each core holds a
1/8 slice of the binarized FC weights) with AllGathers of the (small) sign
activations between layers.

Math: every selu(bn(.)) in the reference is consumed by a binarization
(ste_sign), and selu/bn are monotone, so each binarization collapses to a
per-channel threshold on the raw conv/fc accumulator. All binary layers are
exact integer arithmetic (signs are exactly representable in bf16, matmul
accumulates in fp32 PSUM), so the only approximate layer is conv1, computed
in fp32 on the PE. conv1 (11x11 stride 4) is reshaped host-side into a 3x3
stride-1 conv over 48 phase channels on 57x57 grids.
"""

import os
import sys

sys.path.insert(0, "/opt/trn_rl_repo")

import numpy as np
import ml_dtypes

import concourse.bass as bass
import concourse.tile as tile
import concourse.mybir as mybir
from concourse.vector_clock import ScopedClock
from concourse.bass_utils import run_bass_kernel_spmd

F32 = mybir.dt.float32
BF16 = mybir.dt.bfloat16
AF = mybir.ActivationFunctionType
EPS = 1e-5
NCORES = 8


# ----------------------------------------------------------------------------
# Workaround: this container's walrus build rejects sync-waits attached to the
# CTRL-class Drain instruction Tile emits at TileContext exit ("Too many sync
# wait commands"). Re-emit those waits as standalone single-wait NoOps.
# ----------------------------------------------------------------------------
def _patched_drain_and_barrier(self, tick_clock, wait_clock):
    nc = self.nc
    nop = nc.sync.nop()
    wait_clock.add_sem_waits(nop.ins, ScopedClock({None: tick_clock.global_clock}))
    si = nop.ins.sync_info
    waits = list(si.on_wait) if si is not None else []
    if si is not None:
        si.on_wait = []
    for w in waits:
        ev = nc.sync.nop()
        ev.ins.sync_info = mybir.SyncInfo(on_wait=[w], on_update=[])
    nc.sync.drain()
    nc.all_engine_barrier()
    assert self.sems is not None
    popped = nc._tile_sem_poison_stack.pop()
    assert popped is self._sem_poison
    nc.clear_and_free_semaphores(list(self.sems.allocated().values()))
    nc.all_engine_barrier()


tile.TileContext._drain_and_barrier = _patched_drain_and_barrier


def _peel_excess_waits(nc, limit=1):
    """This walrus build accepts at most ~2 sync waits per instruction (and 1
    on Drain). Move excess waits onto bass_nofuse NoOps inserted immediately
    before the instruction on the same engine."""
    idx = 0
    for f in nc.m.functions:
        for blk in f.blocks:
            new_insts = []
            for inst in blk.instructions:
                si = getattr(inst, "sync_info", None)
                lim = limit
                if si is not None and si.on_wait and len(si.on_wait) > lim:
                    waits = list(si.on_wait)
                    keep = waits[:lim]
                    rest = waits[lim:]
                    while rest:
                        chunk, rest = rest[:limit], rest[limit:]
                        nop = mybir.InstNoOp(
                            name=f"peelw-{idx}",
                            sync_info=mybir.SyncInfo(on_wait=chunk, on_update=[]),
                            bass_nofuse=True,
                            engine=inst.engine,
                        )
                        idx += 1
                        new_insts.append(nop)
                    inst.sync_info = mybir.SyncInfo(
                        on_wait=keep, on_update=list(si.on_update)
                    )
                new_insts.append(inst)
            blk.instructions[:] = new_insts
    return idx


# ----------------------------------------------------------------------------
# Device program
# ----------------------------------------------------------------------------
def build_nc(B=16):
    """Build the per-core Bass program. B = images per core."""
    nc = bass.Bass(trn_type="TRN2", num_devices=NCORES)
    d = {}

    def inp(name, shape, dt):
        d[name] = nc.dram_tensor(name, shape, dt, kind="ExternalInput")
        return d[name]

    xph = inp("xph", [48, B, 57, 57], F32)
    w1f = inp("w1f", [96, 3, 64], F32)
    w1h = inp("w1h", [48, 3, 64], F32)
    w2f = inp("w2f", [128, 10, 192], BF16)
    w2h = inp("w2h", [64, 5, 192], BF16)
    w3a = inp("w3a", [128, 9, 384], BF16)
    w3bf = inp("w3bf", [128, 3, 384], BF16)
    w3bh = inp("w3bh", [64, 3, 384], BF16)
    w4 = inp("w4", [128, 27, 256], BF16)
    w5 = inp("w5", [128, 18, 256], BF16)
    fw1s = inp("fw1s", [72, 128, 512], BF16)
    fw2s = inp("fw2s", [32, 128, 512], BF16)
    fw3s = inp("fw3s", [32, 128, 125], BF16)
    taus = inp("taus", [128, 20], F32)
    idbf = inp("idbf", [128, 128], BF16)
    idf = inp("idf", [128, 128], F32)
    out = nc.dram_tensor("out", [NCORES * B, 1000], F32, kind="ExternalOutput")

    rg = [list(range(NCORES))]

    with tile.TileContext(nc) as tc:
        _build_body(nc, tc, d, out, B, rg)
    _peel_excess_waits(nc)
    return nc


def _build_body(nc, tc, d, out, B, rg):
    from contextlib import ExitStack

    ctx = ExitStack()
    with ctx:
        # ------------------------------------------------------ persistent pools
        singles = ctx.enter_context(tc.tile_pool(name="singles", bufs=1))
        acts = ctx.enter_context(tc.tile_pool(name="acts", bufs=1))
        dram = ctx.enter_context(tc.tile_pool(name="dram", bufs=1, space="DRAM"))

        # weights + constants to SBUF
        def load(name, shape, dt):
            t = singles.tile(shape, dt, name=f"sb_{name}")
            nc.sync.dma_start(out=t, in_=d[name][tuple(slice(None) for _ in shape)])
            return t

        w1f_sb = load("w1f", [96, 3, 64], F32)
        w1h_sb = load("w1h", [48, 3, 64], F32)
        w2f_sb = load("w2f", [128, 10, 192], BF16)
        w2h_sb = load("w2h", [64, 5, 192], BF16)
        w3a_sb = load("w3a", [128, 9, 384], BF16)
        w3bf_sb = load("w3bf", [128, 3, 384], BF16)
        w3bh_sb = load("w3bh", [64, 3, 384], BF16)
        w4_sb = load("w4", [128, 27, 256], BF16)
        w5_sb = load("w5", [128, 18, 256], BF16)
        taus_sb = load("taus", [128, 20], F32)
        idbf_sb = load("idbf", [128, 128], BF16)
        idf_sb = load("idf", [128, 128], F32)

        def ntau(col, p=128):
            return taus_sb[0:p, col : col + 1]

        # activation tiles that live across layer boundaries
        conv2in = acts.tile([128, B, 31, 31], BF16, name="conv2in")
        conv3inA = acts.tile([128, B, 15, 15], BF16, name="conv3inA")
        conv3inB = acts.tile([128, B, 15, 15], BF16, name="conv3inB")
        conv4in = [acts.tile([128, B, 15, 15], BF16, name=f"conv4in{i}") for i in range(3)]
        conv5in = [acts.tile([128, B, 15, 15], BF16, name=f"conv5in{i}") for i in range(2)]
        fc1sign = [acts.tile([128, B, 36], BF16, name=f"fc1sign{i}") for i in range(2)]

        for t in [conv2in, conv3inA, conv3inB] + conv4in + conv5in:
            nc.gpsimd.memset(t[:, :, :, :], 0.0)

        # ---------------------------------------------------------------- conv1
        # phase-reshaped 3x3 stride-1 conv, 48ch -> 64ch, on 57x57 -> 55x55,
        # then maxpool 3s2 -> 27x27, threshold-sign -> conv2in.
        # Images processed in pairs; img A accumulates in PSUM partitions 0:64
        # (PE column-group 0/1), img B in 64:128 (column-group 2/3), which the
        # PE runs concurrently.
        sub_rows = [(r, min(8, 55 - r)) for r in range(0, 55, 8)]  # 7 subtiles
        with tc.tile_pool(name="xpool", bufs=2) as xpool, \
             tc.tile_pool(name="c1ps", bufs=3, space="PSUM") as c1psA, \
             tc.tile_pool(name="c1psB", bufs=3, space="PSUM") as c1psB, \
             tc.tile_pool(name="c1land", bufs=2) as c1land, \
             tc.tile_pool(name="c1pool", bufs=1) as c1pool:
            for pair in range((B + 1) // 2):
                nA, nB = 2 * pair, min(2 * pair + 1, B - 1)
                xp = xpool.tile([96, 2, 57, 57], F32, name="xp")
                nc.sync.dma_start(out=xp[0:48], in_=d["xph"][:, nA : nA + 2])
                nc.sync.dma_start(
                    out=xp[48:96, :, :, 0:56], in_=d["xph"][:, nA : nA + 2, :, 1:57]
                )
                land = c1land.tile([128, 55, 55], F32, name="land")
                for r0, nr in sub_rows:
                    psA = c1psA.tile([128, 440], F32, name="psA")
                    psB = c1psB.tile([128, 440], F32, name="psB")
                    n_mm = 6
                    i = 0
                    for ah in range(3):
                        for half in range(2):
                            st = i == 0
                            sp = i == n_mm - 1
                            if half == 0:
                                lhs = w1f_sb[:, ah, :]
                                rA = xp[0:96, 0, ah + r0 : ah + r0 + nr, 0:55]
                                rB = xp[0:96, 1, ah + r0 : ah + r0 + nr, 0:55]
                            else:
                                lhs = w1h_sb[:, ah, :]
                                rA = xp[0:48, 0, ah + r0 : ah + r0 + nr, 2:57]
                                rB = xp[0:48, 1, ah + r0 : ah + r0 + nr, 2:57]
                            nc.tensor.matmul(
                                psA[0:64, 0 : nr * 55], lhs, rA, start=st, stop=sp
                            )
                            nc.tensor.matmul(
                                psB[64:128, 0 : nr * 55], lhs, rB, start=st, stop=sp
                            )
                            i += 1
                    nc.scalar.activation(
                        land[0:64, r0 : r0 + nr, :],
                        psA[0:64, 0 : nr * 55].rearrange("p (r c) -> p r c", c=55),
                        AF.Copy,
                    )
                    nc.scalar.activation(
                        land[64:128, r0 : r0 + nr, :],
                        psB[64:128, 0 : nr * 55].rearrange("p (r c) -> p r c", c=55),
                        AF.Copy,
                    )
                # maxpool 3x3 stride 2: 55 -> 27 (rows then cols)
                rt = c1pool.tile([128, 27, 55], F32, name="c1rt")
                ct = c1pool.tile([128, 27, 27], F32, name="c1ct")
                nc.vector.tensor_max(rt, land[:, 0:53:2, :], land[:, 1:54:2, :])
                nc.vector.tensor_max(rt, rt, land[:, 2:55:2, :])
                nc.vector.tensor_max(ct, rt[:, :, 0:53:2], rt[:, :, 1:54:2])
                nc.vector.tensor_max(ct, ct, rt[:, :, 2:55:2])
                sg = c1pool.tile([128, 27, 27], BF16, name="c1sg")
                nc.scalar.activation(sg, ct, AF.Sign, bias=ntau(0))
                # de-parity + shifted replica into conv2in (SBUF->SBUF DMA moves
                # data across partitions)
                nc.sync.dma_start(out=conv2in[0:64, nA, 2:29, 2:29], in_=sg[0:64])
                nc.sync.dma_start(out=conv2in[64:128, nA, 2:29, 1:28], in_=sg[0:64])
                if nB != nA:
                    nc.sync.dma_start(out=conv2in[0:64, nB, 2:29, 2:29], in_=sg[64:128])
                    nc.sync.dma_start(out=conv2in[64:128, nB, 2:29, 1:28], in_=sg[64:128])

        # ---------------------------------------------------------------- conv2
        # 5x5 pad 2 on 27x27, 64 -> 192 ch, then pool 27->13, sign -> conv3in.
        # K-chunks: (kh, kw-pair) over [ch | ch col+1] partitions, plus kw=4
        # half chunks.
        with tc.tile_pool(name="c2ps", bufs=4, space="PSUM") as c2ps, \
             tc.tile_pool(name="c2land", bufs=2) as c2land:
            for n in range(B):
                landa = c2land.tile([128, 27, 27], F32, name="landa")
                landb = c2land.tile([128, 27, 27], F32, name="landb")
                for mt, msl, ldst in ((0, slice(0, 128), landa), (1, slice(128, 192), landb)):
                    Mt = msl.stop - msl.start
                    for r0, nr in ((0, 18), (18, 9)):
                        ps = c2ps.tile([128, 486], F32, name="c2p")
                        i = 0
                        for kh in range(5):
                            for kwp in range(2):
                                nc.tensor.matmul(
                                    ps[0:Mt, 0 : nr * 27],
                                    w2f_sb[:, kh * 2 + kwp, msl],
                                    conv2in[:, n, kh + r0 : kh + r0 + nr, 2 * kwp : 2 * kwp + 27],
                                    start=(i == 0), stop=False,
                                )
                                i += 1
                            nc.tensor.matmul(
                                ps[0:Mt, 0 : nr * 27],
                                w2h_sb[:, kh, msl],
                                conv2in[0:64, n, kh + r0 : kh + r0 + nr, 4:31],
                                start=False, stop=(kh == 4),
                            )
                        nc.scalar.activation(
                            ldst[0:Mt, r0 : r0 + nr, :],
                            ps[0:Mt, 0 : nr * 27].rearrange("p (r c) -> p r c", c=27),
                            AF.Copy,
                        )
                # pool 27 -> 13 and sign
                rta = c2land.tile([128, 13, 27], F32, name="c2rta")
                cta = c2land.tile([128, 13, 13], F32, name="c2cta")
                for ldst, P, ntc, dst in (
                    (landa, 128, 1, conv3inA),
                    (landb, 64, 2, conv3inB),
                ):
                    nc.vector.tensor_max(rta[0:P], ldst[0:P, 0:25:2, :], ldst[0:P, 1:26:2, :])
                    nc.vector.tensor_max(rta[0:P], rta[0:P], ldst[0:P, 2:27:2, :])
                    nc.vector.tensor_max(cta[0:P], rta[0:P, :, 0:25:2], rta[0:P, :, 1:26:2])
                    nc.vector.tensor_max(cta[0:P], cta[0:P], rta[0:P, :, 2:27:2])
                    nc.scalar.activation(
                        dst[0:P, n, 1:14, 1:14], cta[0:P], AF.Sign, bias=ntau(ntc, P)
                    )
                # shifted replica of conv3inB's 64 channels
                nc.sync.dma_start(
                    out=conv3inB[64:128, n, :, 0:14], in_=conv3inB[0:64, n, :, 1:15]
                )

        # ------------------------------------------------------------- conv3/4/5
        def conv3x3(n, src_mm, nchunks, mtiles, co_w, psname, post):
            """Emit one image of a 3x3 conv layer. src_mm(i, kh, msl) yields
            (lhsT, rhs) for chunk i; post(mt, ps) consumes the accumulator."""
            for mt in range(mtiles):
                msl = slice(mt * 128, min((mt + 1) * 128, co_w))
                Mt = msl.stop - msl.start
                ps = psname.tile([128, 169], F32, name="cps")
                for i in range(nchunks):
                    lhs, rhs = src_mm(i, msl)
                    nc.tensor.matmul(
                        ps[0:Mt, :], lhs, rhs,
                        start=(i == 0), stop=(i == nchunks - 1),
                    )
                post(mt, Mt, ps)

        with tc.tile_pool(name="c3ps", bufs=4, space="PSUM") as c3ps:
            for n in range(B):
                def src3(i, msl, n=n):
                    if i < 9:  # channels 0-127, 9 offsets
                        kh, kw = divmod(i, 3)
                        return (w3a_sb[:, i, msl],
                                conv3inA[:, n, kh : kh + 13, kw : kw + 13])
                    elif i < 12:  # channels 128-191 (+ col-shift), kw pair (0,1)
                        kh = i - 9
                        return (w3bf_sb[:, kh, msl],
                                conv3inB[:, n, kh : kh + 13, 0:13])
                    else:  # channels 128-191, kw = 2
                        kh = i - 12
                        return (w3bh_sb[:, kh, msl],
                                conv3inB[0:64, n, kh : kh + 13, 2:15])

                def post3(mt, Mt, ps, n=n):
                    nc.scalar.activation(
                        conv4in[mt][0:Mt, n, 1:14, 1:14],
                        ps[0:Mt, :].rearrange("p (r c) -> p r c", c=13),
                        AF.Sign, bias=ntau(3 + mt, Mt),
                    )
                conv3x3(n, src3, 15, 3, 384, c3ps, post3)

            for n in range(B):
                def src4(i, msl, n=n):
                    b, off = divmod(i, 9)
                    kh, kw = divmod(off, 3)
                    return (w4_sb[:, i, msl],
                            conv4in[b][:, n, kh : kh + 13, kw : kw + 13])

                def post4(mt, Mt, ps, n=n):
                    nc.scalar.activation(
                        conv5in[mt][0:Mt, n, 1:14, 1:14],
                        ps[0:Mt, :].rearrange("p (r c) -> p r c", c=13),
                        AF.Sign, bias=ntau(6 + mt, Mt),
                    )
                conv3x3(n, src4, 27, 2, 256, c3ps, post4)

            with tc.tile_pool(name="c5land", bufs=2) as c5land:
                for n in range(B):
                    def src5(i, msl, n=n):
                        b, off = divmod(i, 9)
                        kh, kw = divmod(off, 3)
                        return (w5_sb[:, i, msl],
                                conv5in[b][:, n, kh : kh + 13, kw : kw + 13])

                    def post5(mt, Mt, ps, n=n):
                        land5 = c5land.tile([128, 13, 13], F32, name="land5")
                        rt5 = c5land.tile([128, 6, 13], F32, name="rt5")
                        ct5 = c5land.tile([128, 6, 6], F32, name="ct5")
                        nc.scalar.activation(
                            land5[0:Mt],
                            ps[0:Mt, :].rearrange("p (r c) -> p r c", c=13),
                            AF.Copy,
                        )
                        nc.vector.tensor_max(rt5[0:Mt], land5[0:Mt, 0:11:2, :], land5[0:Mt, 1:12:2, :])
                        nc.vector.tensor_max(rt5[0:Mt], rt5[0:Mt], land5[0:Mt, 2:13:2, :])
                        nc.vector.tensor_max(ct5[0:Mt], rt5[0:Mt, :, 0:11:2], rt5[0:Mt, :, 1:12:2])
                        nc.vector.tensor_max(ct5[0:Mt], ct5[0:Mt], rt5[0:Mt, :, 2:13:2])
                        nc.scalar.activation(
                            fc1sign[mt][0:Mt, n, :],
                            ct5[0:Mt].rearrange("p a b -> p (a b)"),
                            AF.Sign, bias=ntau(8 + mt, Mt),
                        )
                    conv3x3(n, src5, 18, 2, 256, c3ps, post5)

        # ---------------------------------------------------------------- FC 1-3
        # fc1 input allgather: each rank contributes its B images as rows
        # [B, 9216] (feature order = NCHW flatten: ch*36 + r*6 + c).
        ag1_in = dram.tile([B, 9216], BF16, name="ag1_in")
        ag1_out = dram.tile([NCORES * B, 9216], BF16, name="ag1_out", addr_space="Shared")
        for mt in range(2):
            dst = bass.AP(
                tensor=ag1_in.tensor,
                offset=ag1_in.offset + mt * 128 * 36,
                ap=[[36, 128], [9216, B], [1, 36]],
            )
            nc.sync.dma_start(out=dst, in_=fc1sign[mt])
        nc.gpsimd.collective_compute(
            "AllGather", mybir.AluOpType.bypass, replica_groups=rg,
            ins=[ag1_in.opt()], outs=[ag1_out.opt()],
        )

        NB = NCORES * B  # total images
        with tc.tile_pool(name="fcrhs", bufs=1) as fcrhs, \
             tc.tile_pool(name="fctmp", bufs=4) as fctmp, \
             tc.tile_pool(name="fcw", bufs=6) as fcw, \
             tc.tile_pool(name="fcps", bufs=1, space="PSUM") as fcps, \
             tc.tile_pool(name="tps", bufs=2, space="PSUM") as tps:
            # transpose AG1 output into [K, images] chunks
            fc1rhs = fcrhs.tile([128, 72, NB], BF16, name="fc1rhs")
            for k in range(72):
                tin = fctmp.tile([128, 128], BF16, name="tin")
                nc.sync.dma_start(out=tin[0:NB], in_=ag1_out[:, k * 128 : (k + 1) * 128])
                tp = tps.tile([128, 128], BF16, name="tp")
                nc.tensor.transpose(tp[:, 0:NB], tin[0:NB], idbf_sb[0:NB, 0:NB])
                nc.vector.tensor_copy(fc1rhs[:, k, :], tp[:, 0:NB])

            def fclayer(nk, wsb_name, wdram, wwidth, rhs_tile, mwidths, sign_out):
                """shared fc matmul + postprocess structure"""
                pss = [fcps.tile([128, NB], F32, name=f"fcpsum{m}")
                       for m in range(len(mwidths))]
                for k in range(nk):
                    wt = fcw.tile([128, 512], BF16, name="fcwt")
                    nc.sync.dma_start(out=wt[:, 0:wwidth], in_=wdram[k])
                    for m, Mt in enumerate(mwidths):
                        nc.tensor.matmul(
                            pss[m][0:Mt, :],
                            wt[:, m * 128 : m * 128 + Mt],
                            rhs_tile[:, k, :],
                            start=(k == 0), stop=(k == nk - 1),
                        )
                for m, Mt in enumerate(mwidths):
                    sign_out(m, Mt, pss[m])

            # ---- fc1: K=9216 (72 chunks), M=512 slice, N=all images
            ag2_in = dram.tile([512, NB], BF16, name="ag2_in")
            ag2_out = dram.tile([4096, NB], BF16, name="ag2_out", addr_space="Shared")

            def post_fc1(m, Mt, ps):
                sg = fctmp.tile([128, NB], BF16, name="fcsg")
                nc.scalar.activation(sg[0:Mt], ps[0:Mt], AF.Sign, bias=ntau(10 + m, Mt))
                nc.sync.dma_start(out=ag2_in[m * 128 : m * 128 + Mt, :], in_=sg[0:Mt])

            fclayer(72, "f1", d["fw1s"], 512, fc1rhs, [128, 128, 128, 128], post_fc1)
            nc.gpsimd.collective_compute(
                "AllGather", mybir.AluOpType.bypass, replica_groups=rg,
                ins=[ag2_in.opt()], outs=[ag2_out.opt()],
            )

            # ---- fc2
            fc2rhs = fcrhs.tile([128, 32, NB], BF16, name="fc2rhs")
            for k in range(32):
                nc.sync.dma_start(out=fc2rhs[:, k, :], in_=ag2_out[k * 128 : (k + 1) * 128, :])
            ag3_in = dram.tile([512, NB], BF16, name="ag3_in")
            ag3_out = dram.tile([4096, NB], BF16, name="ag3_out", addr_space="Shared")

            def post_fc2(m, Mt, ps):
                sg = fctmp.tile([128, NB], BF16, name="fcsg2")
                nc.scalar.activation(sg[0:Mt], ps[0:Mt], AF.Sign, bias=ntau(14 + m, Mt))
                nc.sync.dma_start(out=ag3_in[m * 128 : m * 128 + Mt, :], in_=sg[0:Mt])

            fclayer(32, "f2", d["fw2s"], 512, fc2rhs, [128, 128, 128, 128], post_fc2)
            nc.gpsimd.collective_compute(
                "AllGather", mybir.AluOpType.bypass, replica_groups=rg,
                ins=[ag3_in.opt()], outs=[ag3_out.opt()],
            )

            # ---- fc3 + bn8 affine
            fc3rhs = fcrhs.tile([128, 32, NB], BF16, name="fc3rhs")
            for k in range(32):
                nc.sync.dma_start(out=fc3rhs[:, k, :], in_=ag3_out[k * 128 : (k + 1) * 128, :])
            ag4_in = dram.tile([125, NB], F32, name="ag4_in")
            ag4_out = dram.tile([1000, NB], F32, name="ag4_out", addr_space="Shared")

            def post_fc3(m, Mt, ps):
                lg = fctmp.tile([128, NB], F32, name="fclg")
                nc.vector.tensor_scalar(
                    lg[0:Mt], ps[0:Mt],
                    taus_sb[0:Mt, 18:19], taus_sb[0:Mt, 19:20],
                    op0=mybir.AluOpType.mult, op1=mybir.AluOpType.add,
                )
                nc.sync.dma_start(out=ag4_in[:, :], in_=lg[0:Mt])

            fclayer(32, "f3", d["fw3s"], 125, fc3rhs, [125], post_fc3)
            nc.gpsimd.collective_compute(
                "AllGather", mybir.AluOpType.bypass, replica_groups=rg,
                ins=[ag4_in.opt()], outs=[ag4_out.opt()],
            )

            # ---- transpose logits to [images, 1000] and log_softmax
            logt = fcrhs.tile([128, 1000], F32, name="logt")
            for t in range(8):
                lin = fctmp.tile([128, NB], F32, name="lin")
                nc.sync.dma_start(out=lin[0:125], in_=ag4_out[t * 125 : (t + 1) * 125, :])
                ltp = tps.tile([128, 128], F32, name="ltp")
                nc.tensor.transpose(ltp[0:NB, 0:125], lin[0:125, 0:NB], idf_sb[0:125, 0:125])
                nc.scalar.activation(
                    logt[0:NB, t * 125 : (t + 1) * 125], ltp[0:NB, 0:125], AF.Copy
                )
            mx = fctmp.tile([128, 1], F32, name="mx", bufs=1)
            nmx = fctmp.tile([128, 1], F32, name="nmx", bufs=1)
            sh = fctmp.tile([128, 1000], F32, name="sh", bufs=1)
            ex = fctmp.tile([128, 1000], F32, name="ex", bufs=1)
            sm = fctmp.tile([128, 1], F32, name="sm", bufs=1)
            lns = fctmp.tile([128, 1], F32, name="lns", bufs=1)
            res = fctmp.tile([128, 1000], F32, name="res", bufs=1)
            nc.vector.tensor_reduce(mx[0:NB], logt[0:NB], axis=mybir.AxisListType.X,
                                    op=mybir.AluOpType.max)
            nc.vector.tensor_scalar_mul(nmx[0:NB], mx[0:NB], -1.0)
            nc.vector.tensor_scalar(sh[0:NB], logt[0:NB], nmx[0:NB], -85.0,
                                    op0=mybir.AluOpType.add, op1=mybir.AluOpType.max)
            nc.scalar.activation(ex[0:NB], sh[0:NB], AF.Exp)
            nc.vector.tensor_reduce(sm[0:NB], ex[0:NB], axis=mybir.AxisListType.X,
                                    op=mybir.AluOpType.add)
            nc.scalar.activation(lns[0:NB], sm[0:NB], AF.Ln)
            nc.vector.tensor_scalar(res[0:NB], logt[0:NB], nmx[0:NB], lns[0:NB],
                                    op0=mybir.AluOpType.add, op1=mybir.AluOpType.subtract)
            nc.sync.dma_start(out=out[:, :], in_=res[0:NB])


# ----------------------------------------------------------------------------
# Host-side preparation
# ----------------------------------------------------------------------------
def _sgn(a):
    return np.where(a >= 0, np.float32(1.0), np.float32(-1.0))


def prep_inputs(inputs, B=16):
    """Full-batch inputs -> per-core in_maps."""
    bf = ml_dtypes.bfloat16
    x = np.asarray(inputs["x"], np.float32)
    NB = x.shape[0]
    assert NB == NCORES * B

    xp = np.zeros((NB, 3, 228, 228), np.float32)
    xp[:, :, 2:226, 2:226] = x
    xph = (
        xp.reshape(NB, 3, 57, 4, 57, 4)
        .transpose(0, 1, 3, 5, 2, 4)
        .reshape(NB, 48, 57, 57)
    )

    wb1 = _sgn(np.asarray(inputs["cw1"], np.float32))  # [64,3,11,11]
    w1p = np.zeros((48, 3, 3, 64), np.float32)
    for ci in range(3):
        for kh in range(11):
            ah, rh = divmod(kh, 4)
            for kw in range(11):
                aw, rw = divmod(kw, 4)
                w1p[ci * 16 + rh * 4 + rw, ah, aw, :] = wb1[:, ci, kh, kw]
    w1f = np.concatenate([w1p[:, :, 0, :], w1p[:, :, 1, :]], axis=0)  # [96,3,64]
    w1h = np.ascontiguousarray(w1p[:, :, 2, :])  # [48,3,64]

    wb2 = _sgn(np.asarray(inputs["cw2"], np.float32))  # [192,64,5,5]
    w2f = np.zeros((128, 10, 192), np.float32)
    w2h = np.zeros((64, 5, 192), np.float32)
    for kh in range(5):
        for p_ in range(2):
            kw = 2 * p_
            w2f[0:64, kh * 2 + p_, :] = wb2[:, :, kh, kw].T
            w2f[64:128, kh * 2 + p_, :] = wb2[:, :, kh, kw + 1].T
        w2h[:, kh, :] = wb2[:, :, kh, 4].T

    wb3 = _sgn(np.asarray(inputs["cw3"], np.float32))  # [384,192,3,3]
    w3a = np.zeros((128, 9, 384), np.float32)
    w3bf = np.zeros((128, 3, 384), np.float32)
    w3bh = np.zeros((64, 3, 384), np.float32)
    for kh in range(3):
        for kw in range(3):
            w3a[:, kh * 3 + kw, :] = wb3[:, 0:128, kh, kw].T
        w3bf[0:64, kh, :] = wb3[:, 128:192, kh, 0].T
        w3bf[64:128, kh, :] = wb3[:, 128:192, kh, 1].T
        w3bh[:, kh, :] = wb3[:, 128:192, kh, 2].T

    wb4 = _sgn(np.asarray(inputs["cw4"], np.float32))  # [256,384,3,3]
    w4 = np.zeros((128, 27, 256), np.float32)
    for b in range(3):
        for kh in range(3):
            for kw in range(3):
                w4[:, b * 9 + kh * 3 + kw, :] = wb4[:, b * 128 : (b + 1) * 128, kh, kw].T
    wb5 = _sgn(np.asarray(inputs["cw5"], np.float32))  # [256,256,3,3]
    w5 = np.zeros((128, 18, 256), np.float32)
    for b in range(2):
        for kh in range(3):
            for kw in range(3):
                w5[:, b * 9 + kh * 3 + kw, :] = wb5[:, b * 128 : (b + 1) * 128, kh, kw].T

    FW1 = _sgn(np.asarray(inputs["fw1"], np.float32))  # [4096, 9216]
    FW2 = _sgn(np.asarray(inputs["fw2"], np.float32))  # [4096, 4096]
    FW3 = _sgn(np.asarray(inputs["fw3"], np.float32))  # [1000, 4096]

    def tau(g, be, m, v, bias):
        g = np.asarray(g, np.float64)
        assert (g > 0).all(), "bn gamma must be positive for the sign fold"
        t = (np.asarray(m, np.float64)
             - np.asarray(be, np.float64) * np.sqrt(np.asarray(v, np.float64) + EPS) / g
             - np.asarray(bias, np.float64))
        return t.astype(np.float32)

    i = inputs
    t1 = tau(i["g1"], i["be1"], i["m1"], i["v1"], i["cb1"])
    t2 = tau(i["g2"], i["be2"], i["m2"], i["v2"], i["cb2"])
    t3 = tau(i["g3"], i["be3"], i["m3"], i["v3"], i["cb3"])
    t4 = tau(i["g4"], i["be4"], i["m4"], i["v4"], i["cb4"])
    t5 = tau(i["g5"], i["be5"], i["m5"], i["v5"], i["cb5"])
    t6 = tau(i["g6"], i["be6"], i["m6"], i["v6"], i["fb1"])
    t7 = tau(i["g7"], i["be7"], i["m7"], i["v7"], i["fb2"])
    g8 = np.asarray(i["g8"], np.float64)
    s8 = np.sqrt(np.asarray(i["v8"], np.float64) + EPS)
    scale8 = (g8 / s8).astype(np.float32)
    bias8 = ((np.asarray(i["fb3"], np.float64) - np.asarray(i["m8"], np.float64)) * g8 / s8
             + np.asarray(i["be8"], np.float64)).astype(np.float32)

    idbf = np.eye(128, dtype=bf)
    idf = np.eye(128, dtype=np.float32)

    shared = {
        "w1f": w1f, "w1h": w1h,
        "w2f": w2f.astype(bf), "w2h": w2h.astype(bf),
        "w3a": w3a.astype(bf), "w3bf": w3bf.astype(bf), "w3bh": w3bh.astype(bf),
        "w4": w4.astype(bf), "w5": w5.astype(bf),
        "idbf": idbf, "idf": idf,
    }

    in_maps = []
    for r in range(NCORES):
        taus_t = np.zeros((128, 20), np.float32)
        taus_t[:, 0] = -np.concatenate([t1, t1])
        taus_t[:, 1] = -t2[0:128]
        taus_t[0:64, 2] = -t2[128:192]
        for m in range(3):
            taus_t[:, 3 + m] = -t3[m * 128 : (m + 1) * 128]
        for m in range(2):
            taus_t[:, 6 + m] = -t4[m * 128 : (m + 1) * 128]
            taus_t[:, 8 + m] = -t5[m * 128 : (m + 1) * 128]
        for m in range(4):
            taus_t[:, 10 + m] = -t6[r * 512 + m * 128 : r * 512 + (m + 1) * 128]
            taus_t[:, 14 + m] = -t7[r * 512 + m * 128 : r * 512 + (m + 1) * 128]
        taus_t[0:125, 18] = scale8[r * 125 : (r + 1) * 125]
        taus_t[0:125, 19] = bias8[r * 125 : (r + 1) * 125]

        fw1s = np.ascontiguousarray(
            FW1[r * 512 : (r + 1) * 512, :].T.reshape(72, 128, 512)
        ).astype(bf)
        fw2s = np.ascontiguousarray(
            FW2[r * 512 : (r + 1) * 512, :].T.reshape(32, 128, 512)
        ).astype(bf)
        fw3s = np.ascontiguousarray(
            FW3[r * 125 : (r + 1) * 125, :].T.reshape(32, 128, 125)
        ).astype(bf)

        im = dict(shared)
        im["xph"] = np.ascontiguousarray(
            xph[r * B : (r + 1) * B].transpose(1, 0, 2, 3)
        )
        im["taus"] = taus_t
        im["fw1s"] = fw1s
        im["fw2s"] = fw2s
        im["fw3s"] = fw3s
        in_maps.append(im)
    return in_maps


_NC_CACHE = {}


def run(inputs, B=16, trace=False, **kw):
    if B not in _NC_CACHE:
        _NC_CACHE[B] = build_nc(B)
    nc = _NC_CACHE[B]
    in_maps = prep_inputs(inputs, B)
    res = run_bass_kernel_spmd(
        nc, in_maps, core_ids=list(range(NCORES)), trace=trace, **kw
    )
    return res.results[0]["out"].astype(np.float32), res


def kernel(**inputs) -> np.ndarray:
    out, _ = run(inputs, B=16)
    return out


# revision 18
# speedup vs baseline: 1.0257x; 1.0257x over previous
"""Binary-AlexNet forward pass on 8 Trainium2 NeuronCores (Bass/Tile).

Strategy
--------
Data parallel over the batch: each of the 8 cores runs 16 images through the
conv stack; the fully-connected layers are model-parallel (each core holds a
1/8 slice of the binarized FC weights) with AllGathers of the (small) sign
activations between layers.

Math: every selu(bn(.)) in the reference is consumed by a binarization
(ste_sign), and selu/bn are monotone, so each binarization collapses to a
per-channel threshold on the raw conv/fc accumulator. All binary layers are
exact integer arithmetic (signs are exactly representable in bf16, matmul
accumulates in fp32 PSUM), so the only approximate layer is conv1, computed
in fp32 on the PE. conv1 (11x11 stride 4) is reshaped host-side into a 3x3
stride-1 conv over 48 phase channels on 57x57 grids.
"""

import os
import sys

sys.path.insert(0, "/opt/trn_rl_repo")

import numpy as np
import ml_dtypes

import concourse.bass as bass
import concourse.tile as tile
import concourse.mybir as mybir
from concourse.vector_clock import ScopedClock
from concourse.bass_utils import run_bass_kernel_spmd

F32 = mybir.dt.float32
BF16 = mybir.dt.bfloat16
AF = mybir.ActivationFunctionType
EPS = 1e-5
NCORES = 8


# ----------------------------------------------------------------------------
# Workaround: this container's walrus build rejects sync-waits attached to the
# CTRL-class Drain instruction Tile emits at TileContext exit ("Too many sync
# wait commands"). Re-emit those waits as standalone single-wait NoOps.
# ----------------------------------------------------------------------------
def _patched_drain_and_barrier(self, tick_clock, wait_clock):
    nc = self.nc
    nop = nc.sync.nop()
    wait_clock.add_sem_waits(nop.ins, ScopedClock({None: tick_clock.global_clock}))
    si = nop.ins.sync_info
    waits = list(si.on_wait) if si is not None else []
    if si is not None:
        si.on_wait = []
    for w in waits:
        ev = nc.sync.nop()
        ev.ins.sync_info = mybir.SyncInfo(on_wait=[w], on_update=[])
    nc.sync.drain()
    nc.all_engine_barrier()
    assert self.sems is not None
    popped = nc._tile_sem_poison_stack.pop()
    assert popped is self._sem_poison
    nc.clear_and_free_semaphores(list(self.sems.allocated().values()))
    nc.all_engine_barrier()


tile.TileContext._drain_and_barrier = _patched_drain_and_barrier


def _enable_ldw_opt():
    """Compile with walrus --enable-ldw-opt=true (weight-load double
    buffering). Requires no explicit InstLdweights in the BIR."""
    import concourse.bass_utils as bu
    if getattr(bu.run_command, "_ldw_patched", False):
        return
    orig = bu.run_command

    def patched(argv, **kw):
        argv = ["--enable-ldw-opt=true" if a == "--enable-ldw-opt=false" else a
                for a in argv]
        return orig(argv, **kw)

    patched._ldw_patched = True
    bu.run_command = patched


def _fuse_ldweights(nc):
    """Delete explicit InstLdweights (Tile lowering emits LDW+self-loading-MM
    pairs; the matmul still carries the weights operand). Their sync waits
    move onto the following matmul; _peel_excess_waits handles overflow."""
    removed = 0
    for f in nc.m.functions:
        for blk in f.blocks:
            insts = blk.instructions
            new_insts = []
            pend_waits = []
            for inst in insts:
                tn = type(inst).__name__
                if tn == "InstLdweights":
                    si = getattr(inst, "sync_info", None)
                    if si is not None and si.on_wait:
                        pend_waits.extend(si.on_wait)
                    removed += 1
                    continue
                if pend_waits and tn == "InstMatmult":
                    si = inst.sync_info
                    ow = list(si.on_wait) if si is not None else []
                    ou = list(si.on_update) if si is not None else []
                    inst.sync_info = mybir.SyncInfo(
                        on_wait=pend_waits + ow, on_update=ou
                    )
                    pend_waits = []
                new_insts.append(inst)
            assert not pend_waits, "dangling ldweights waits"
            blk.instructions[:] = new_insts
    return removed


def _peel_excess_waits(nc, limit=1):
    """This walrus build accepts at most ~2 sync waits per instruction (and 1
    on Drain). Move excess waits onto bass_nofuse NoOps inserted immediately
    before the instruction on the same engine."""
    idx = 0
    for f in nc.m.functions:
        for blk in f.blocks:
            new_insts = []
            for inst in blk.instructions:
                si = getattr(inst, "sync_info", None)
                lim = limit
                if si is not None and si.on_wait and len(si.on_wait) > lim:
                    waits = list(si.on_wait)
                    keep = waits[:lim]
                    rest = waits[lim:]
                    while rest:
                        chunk, rest = rest[:limit], rest[limit:]
                        nop = mybir.InstNoOp(
                            name=f"peelw-{idx}",
                            sync_info=mybir.SyncInfo(on_wait=chunk, on_update=[]),
                            bass_nofuse=True,
                            engine=inst.engine,
                        )
                        idx += 1
                        try:
                            nc.register_instruction(nop, overwrite=True)
                        except Exception:
                            pass
                        new_insts.append(nop)
                    inst.sync_info = mybir.SyncInfo(
                        on_wait=keep, on_update=list(si.on_update)
                    )
                new_insts.append(inst)
            blk.instructions[:] = new_insts
    return idx


# ----------------------------------------------------------------------------
# Device program
# ----------------------------------------------------------------------------
def build_nc(B=16, fuse_ldw=False):
    """Build the per-core Bass program. B = images per core."""
    nc = bass.Bass(trn_type="TRN2", num_devices=NCORES)
    d = {}

    def inp(name, shape, dt):
        d[name] = nc.dram_tensor(name, shape, dt, kind="ExternalInput")
        return d[name]

    xph = inp("xph", [48, B, 57, 57], F32)
    w1f = inp("w1f", [96, 3, 64], F32)
    w1h = inp("w1h", [48, 3, 64], F32)
    w2f = inp("w2f", [128, 10, 192], BF16)
    w2h = inp("w2h", [64, 5, 192], BF16)
    w3a = inp("w3a", [128, 9, 384], BF16)
    w3bf = inp("w3bf", [128, 3, 384], BF16)
    w3bh = inp("w3bh", [64, 3, 384], BF16)
    w4 = inp("w4", [128, 27, 256], BF16)
    w5 = inp("w5", [128, 18, 256], BF16)
    fw1s = inp("fw1s", [72, 128, 512], BF16)
    fw2s = inp("fw2s", [32, 128, 512], BF16)
    fw3s = inp("fw3s", [32, 128, 125], BF16)
    taus = inp("taus", [128, 20], F32)
    idbf = inp("idbf", [128, 128], BF16)
    idf = inp("idf", [128, 128], F32)
    out = nc.dram_tensor("out", [NCORES * B, 1000], F32, kind="ExternalOutput")

    rg = [list(range(NCORES))]

    with tile.TileContext(nc) as tc:
        _build_body(nc, tc, d, out, B, rg)
    if fuse_ldw:
        _fuse_ldweights(nc)
    _peel_excess_waits(nc)
    return nc


def _build_body(nc, tc, d, out, B, rg):
    from contextlib import ExitStack

    ctx = ExitStack()
    with ctx:
        # ------------------------------------------------------ persistent pools
        singles = ctx.enter_context(tc.tile_pool(name="singles", bufs=1))
        acts = ctx.enter_context(tc.tile_pool(name="acts", bufs=1))
        dram = ctx.enter_context(tc.tile_pool(name="dram", bufs=1, space="DRAM"))

        # weights + constants to SBUF
        def load(name, shape, dt):
            t = singles.tile(shape, dt, name=f"sb_{name}")
            nc.sync.dma_start(out=t, in_=d[name][tuple(slice(None) for _ in shape)])
            return t

        w1f_sb = load("w1f", [96, 3, 64], F32)
        w1h_sb = load("w1h", [48, 3, 64], F32)
        w2f_sb = load("w2f", [128, 10, 192], BF16)
        w2h_sb = load("w2h", [64, 5, 192], BF16)
        w3a_sb = load("w3a", [128, 9, 384], BF16)
        w3bf_sb = load("w3bf", [128, 3, 384], BF16)
        w3bh_sb = load("w3bh", [64, 3, 384], BF16)
        w4_sb = load("w4", [128, 27, 256], BF16)
        w5_sb = load("w5", [128, 18, 256], BF16)
        taus_sb = load("taus", [128, 20], F32)
        idbf_sb = load("idbf", [128, 128], BF16)
        idf_sb = load("idf", [128, 128], F32)

        def ntau(col, p=128):
            return taus_sb[0:p, col : col + 1]

        # activation tiles that live across layer boundaries
        conv2in = acts.tile([128, B, 31, 31], BF16, name="conv2in")
        conv3inA = acts.tile([128, B, 15, 15], BF16, name="conv3inA")
        conv3inB = acts.tile([128, B, 15, 15], BF16, name="conv3inB")
        conv4in = [acts.tile([128, B, 15, 15], BF16, name=f"conv4in{i}") for i in range(3)]
        conv5in = [acts.tile([128, B, 15, 15], BF16, name=f"conv5in{i}") for i in range(2)]
        fc1sign = [acts.tile([128, B, 36], BF16, name=f"fc1sign{i}") for i in range(2)]

        for t in [conv2in, conv3inA, conv3inB] + conv4in + conv5in:
            nc.gpsimd.memset(t[:, :, :, :], 0.0)

        # ---------------------------------------------------------------- conv1
        # phase-reshaped 3x3 stride-1 conv, 48ch -> 64ch, on 57x57 -> 55x55,
        # then maxpool 3s2 -> 27x27, threshold-sign -> conv2in.
        # Images processed in pairs; img A accumulates in PSUM partitions 0:64
        # (PE column-group 0/1), img B in 64:128 (column-group 2/3), which the
        # PE runs concurrently.
        sub_rows = [(r, min(8, 55 - r)) for r in range(0, 55, 8)]  # 7 subtiles
        with tc.tile_pool(name="xpool", bufs=2) as xpool, \
             tc.tile_pool(name="c1ps", bufs=3, space="PSUM") as c1psA, \
             tc.tile_pool(name="c1psB", bufs=3, space="PSUM") as c1psB, \
             tc.tile_pool(name="c1land", bufs=2) as c1land, \
             tc.tile_pool(name="c1pool", bufs=1) as c1pool:
            for pair in range((B + 1) // 2):
                nA, nB = 2 * pair, min(2 * pair + 1, B - 1)
                xp = xpool.tile([96, 2, 57, 57], F32, name="xp")
                nc.sync.dma_start(out=xp[0:48], in_=d["xph"][:, nA : nA + 2])
                nc.sync.dma_start(
                    out=xp[48:96, :, :, 0:56], in_=d["xph"][:, nA : nA + 2, :, 1:57]
                )
                land = c1land.tile([128, 55, 55], F32, name="land")
                for r0, nr in sub_rows:
                    psA = c1psA.tile([128, 440], F32, name="psA")
                    psB = c1psB.tile([128, 440], F32, name="psB")
                    n_mm = 6
                    i = 0
                    for ah in range(3):
                        for half in range(2):
                            st = i == 0
                            sp = i == n_mm - 1
                            if half == 0:
                                lhs = w1f_sb[:, ah, :]
                                rA = xp[0:96, 0, ah + r0 : ah + r0 + nr, 0:55]
                                rB = xp[0:96, 1, ah + r0 : ah + r0 + nr, 0:55]
                            else:
                                lhs = w1h_sb[:, ah, :]
                                rA = xp[0:48, 0, ah + r0 : ah + r0 + nr, 2:57]
                                rB = xp[0:48, 1, ah + r0 : ah + r0 + nr, 2:57]
                            nc.tensor.matmul(
                                psA[0:64, 0 : nr * 55], lhs, rA, start=st, stop=sp
                            )
                            nc.tensor.matmul(
                                psB[64:128, 0 : nr * 55], lhs, rB, start=st, stop=sp
                            )
                            i += 1
                    nc.scalar.activation(
                        land[0:64, r0 : r0 + nr, :],
                        psA[0:64, 0 : nr * 55].rearrange("p (r c) -> p r c", c=55),
                        AF.Copy,
                    )
                    nc.scalar.activation(
                        land[64:128, r0 : r0 + nr, :],
                        psB[64:128, 0 : nr * 55].rearrange("p (r c) -> p r c", c=55),
                        AF.Copy,
                    )
                # maxpool 3x3 stride 2: 55 -> 27 (rows then cols)
                rt = c1pool.tile([128, 27, 55], F32, name="c1rt")
                ct = c1pool.tile([128, 27, 27], F32, name="c1ct")
                nc.vector.tensor_max(rt, land[:, 0:53:2, :], land[:, 1:54:2, :])
                nc.vector.tensor_max(rt, rt, land[:, 2:55:2, :])
                nc.vector.tensor_max(ct, rt[:, :, 0:53:2], rt[:, :, 1:54:2])
                nc.vector.tensor_max(ct, ct, rt[:, :, 2:55:2])
                sg = c1pool.tile([128, 27, 27], BF16, name="c1sg")
                nc.scalar.activation(sg, ct, AF.Sign, bias=ntau(0))
                # de-parity + shifted replica into conv2in (SBUF->SBUF DMA moves
                # data across partitions)
                nc.sync.dma_start(out=conv2in[0:64, nA, 2:29, 2:29], in_=sg[0:64])
                nc.sync.dma_start(out=conv2in[64:128, nA, 2:29, 1:28], in_=sg[0:64])
                if nB != nA:
                    nc.sync.dma_start(out=conv2in[0:64, nB, 2:29, 2:29], in_=sg[64:128])
                    nc.sync.dma_start(out=conv2in[64:128, nB, 2:29, 1:28], in_=sg[64:128])

        # ---------------------------------------------------------------- conv2
        # 5x5 pad 2 on 27x27, 64 -> 192 ch, then pool 27->13, sign -> conv3in.
        # K-chunks: (kh, kw-pair) over [ch | ch col+1] partitions, plus kw=4
        # half chunks.
        with tc.tile_pool(name="c2ps", bufs=6, space="PSUM") as c2ps, \
             tc.tile_pool(name="c2land", bufs=2) as c2land:
            for n in range(B):
                landa = c2land.tile([128, 27, 27], F32, name="landa")
                landb = c2land.tile([128, 27, 27], F32, name="landb")
                for mt, msl, ldst in ((0, slice(0, 128), landa), (1, slice(128, 192), landb)):
                    Mt = msl.stop - msl.start
                    for r0, nr in ((0, 18), (18, 9)):
                        ps = c2ps.tile([128, 486], F32, name="c2p")
                        i = 0
                        for kh in range(5):
                            for kwp in range(2):
                                nc.tensor.matmul(
                                    ps[0:Mt, 0 : nr * 27],
                                    w2f_sb[:, kh * 2 + kwp, msl],
                                    conv2in[:, n, kh + r0 : kh + r0 + nr, 2 * kwp : 2 * kwp + 27],
                                    start=(i == 0), stop=False,
                                )
                                i += 1
                            nc.tensor.matmul(
                                ps[0:Mt, 0 : nr * 27],
                                w2h_sb[:, kh, msl],
                                conv2in[0:64, n, kh + r0 : kh + r0 + nr, 4:31],
                                start=False, stop=(kh == 4),
                            )
                        nc.scalar.activation(
                            ldst[0:Mt, r0 : r0 + nr, :],
                            ps[0:Mt, 0 : nr * 27].rearrange("p (r c) -> p r c", c=27),
                            AF.Copy,
                        )
                # pool 27 -> 13 and sign
                rta = c2land.tile([128, 13, 27], F32, name="c2rta")
                cta = c2land.tile([128, 13, 13], F32, name="c2cta")
                for ldst, P, ntc, dst in (
                    (landa, 128, 1, conv3inA),
                    (landb, 64, 2, conv3inB),
                ):
                    nc.vector.tensor_max(rta[0:P], ldst[0:P, 0:25:2, :], ldst[0:P, 1:26:2, :])
                    nc.vector.tensor_max(rta[0:P], rta[0:P], ldst[0:P, 2:27:2, :])
                    nc.vector.tensor_max(cta[0:P], rta[0:P, :, 0:25:2], rta[0:P, :, 1:26:2])
                    nc.vector.tensor_max(cta[0:P], cta[0:P], rta[0:P, :, 2:27:2])
                    nc.scalar.activation(
                        dst[0:P, n, 1:14, 1:14], cta[0:P], AF.Sign, bias=ntau(ntc, P)
                    )
                # shifted replica of conv3inB's 64 channels
                nc.sync.dma_start(
                    out=conv3inB[64:128, n, :, 0:14], in_=conv3inB[0:64, n, :, 1:15]
                )

        # ------------------------------------------------------------- conv3/4/5
        # Images are processed in groups of 3 per matmul (N = 3*169 = 507) to
        # amortize weight loads and instruction overhead.
        GI = 3
        groups = [(n0, min(GI, B - n0)) for n0 in range(0, B, GI)]

        def conv3x3(n0, g, mt_list, src_mm, nchunks, co_w, psname, post):
            for mt in mt_list:
                msl = slice(mt * 128, min((mt + 1) * 128, co_w))
                Mt = msl.stop - msl.start
                ps = psname.tile([128, 512], F32, name="cps")
                for i in range(nchunks):
                    lhs, rhs = src_mm(i, msl)
                    nc.tensor.matmul(
                        ps[0:Mt, 0 : g * 169], lhs, rhs,
                        start=(i == 0), stop=(i == nchunks - 1),
                    )
                post(mt, Mt, ps)

        with tc.tile_pool(name="c3ps", bufs=6, space="PSUM") as c3ps:
            for n0, g in groups:
                def src3(i, msl, n0=n0, g=g):
                    if i < 9:  # channels 0-127, 9 offsets
                        kh, kw = divmod(i, 3)
                        return (w3a_sb[:, i, msl],
                                conv3inA[:, n0 : n0 + g, kh : kh + 13, kw : kw + 13])
                    elif i < 12:  # channels 128-191 (+ col-shift), kw pair (0,1)
                        kh = i - 9
                        return (w3bf_sb[:, kh, msl],
                                conv3inB[:, n0 : n0 + g, kh : kh + 13, 0:13])
                    else:  # channels 128-191, kw = 2
                        kh = i - 12
                        return (w3bh_sb[:, kh, msl],
                                conv3inB[0:64, n0 : n0 + g, kh : kh + 13, 2:15])

                def post3(mt, Mt, ps, n0=n0, g=g):
                    nc.scalar.activation(
                        conv4in[mt][0:Mt, n0 : n0 + g, 1:14, 1:14],
                        ps[0:Mt, 0 : g * 169].rearrange("p (n r c) -> p n r c", r=13, c=13),
                        AF.Sign, bias=ntau(3 + mt, Mt),
                    )
                conv3x3(n0, g, range(3), src3, 15, 384, c3ps, post3)

            for n0, g in groups:
                def src4(i, msl, n0=n0, g=g):
                    b, off = divmod(i, 9)
                    kh, kw = divmod(off, 3)
                    return (w4_sb[:, i, msl],
                            conv4in[b][:, n0 : n0 + g, kh : kh + 13, kw : kw + 13])

                def post4(mt, Mt, ps, n0=n0, g=g):
                    nc.scalar.activation(
                        conv5in[mt][0:Mt, n0 : n0 + g, 1:14, 1:14],
                        ps[0:Mt, 0 : g * 169].rearrange("p (n r c) -> p n r c", r=13, c=13),
                        AF.Sign, bias=ntau(6 + mt, Mt),
                    )
                conv3x3(n0, g, range(2), src4, 27, 256, c3ps, post4)

            # conv5: mt-outer so each half's fc1 activations can allgather
            # while the other half still computes.
            ag1_in = [dram.tile([B, 4608], BF16, name=f"ag1_in{m}") for m in range(2)]
            ag1_out = [
                dram.tile([NCORES * B, 4608], BF16, name=f"ag1_out{m}", addr_space="Shared")
                for m in range(2)
            ]
            with tc.tile_pool(name="c5land", bufs=2) as c5land:
                for mt in range(2):
                    for n0, g in groups:
                        def src5(i, msl, n0=n0, g=g):
                            b, off = divmod(i, 9)
                            kh, kw = divmod(off, 3)
                            return (w5_sb[:, i, msl],
                                    conv5in[b][:, n0 : n0 + g, kh : kh + 13, kw : kw + 13])

                        def post5(mt, Mt, ps, n0=n0, g=g):
                            land5 = c5land.tile([128, GI, 13, 13], F32, name="land5")
                            rt5 = c5land.tile([128, GI, 6, 13], F32, name="rt5")
                            ct5 = c5land.tile([128, GI, 6, 6], F32, name="ct5")
                            nc.scalar.activation(
                                land5[0:Mt, 0:g],
                                ps[0:Mt, 0 : g * 169].rearrange("p (n r c) -> p n r c", r=13, c=13),
                                AF.Copy,
                            )
                            nc.vector.tensor_max(rt5[0:Mt, 0:g], land5[0:Mt, 0:g, 0:11:2, :], land5[0:Mt, 0:g, 1:12:2, :])
                            nc.vector.tensor_max(rt5[0:Mt, 0:g], rt5[0:Mt, 0:g], land5[0:Mt, 0:g, 2:13:2, :])
                            nc.vector.tensor_max(ct5[0:Mt, 0:g], rt5[0:Mt, 0:g, :, 0:11:2], rt5[0:Mt, 0:g, :, 1:12:2])
                            nc.vector.tensor_max(ct5[0:Mt, 0:g], ct5[0:Mt, 0:g], rt5[0:Mt, 0:g, :, 2:13:2])
                            nc.scalar.activation(
                                fc1sign[mt][0:Mt, n0 : n0 + g, :],
                                ct5[0:Mt, 0:g].rearrange("p n a b -> p n (a b)"),
                                AF.Sign, bias=ntau(8 + mt, Mt),
                            )
                        conv3x3(n0, g, [mt], src5, 18, 256, c3ps, post5)
                    # fire this half's allgather as soon as its signs are done
                    dst = bass.AP(
                        tensor=ag1_in[mt].tensor,
                        offset=ag1_in[mt].offset,
                        ap=[[36, 128], [4608, B], [1, 36]],
                    )
                    nc.sync.dma_start(out=dst, in_=fc1sign[mt])
                    nc.gpsimd.collective_compute(
                        "AllGather", mybir.AluOpType.bypass, replica_groups=rg,
                        ins=[ag1_in[mt].opt()], outs=[ag1_out[mt].opt()],
                    )

        NB = NCORES * B  # total images
        with tc.tile_pool(name="fcrhs", bufs=1) as fcrhs, \
             tc.tile_pool(name="fctmp", bufs=4) as fctmp, \
             tc.tile_pool(name="fcw", bufs=6) as fcw, \
             tc.tile_pool(name="fcps", bufs=1, space="PSUM") as fcps, \
             tc.tile_pool(name="tps", bufs=2, space="PSUM") as tps:
            # transpose AG1 output into [K, images] chunks
            fc1rhs = fcrhs.tile([128, 72, NB], BF16, name="fc1rhs")
            for k in range(72):
                tin = fctmp.tile([128, 128], BF16, name="tin")
                src = ag1_out[k // 36][:, (k % 36) * 128 : (k % 36 + 1) * 128]
                nc.sync.dma_start(out=tin[0:NB], in_=src)
                tp = tps.tile([128, 128], BF16, name="tp")
                nc.tensor.transpose(tp[:, 0:NB], tin[0:NB], idbf_sb[0:NB, 0:NB])
                nc.vector.tensor_copy(fc1rhs[:, k, :], tp[:, 0:NB])

            def fclayer(nk, wsb_name, wdram, wwidth, rhs_tile, mwidths, sign_out):
                """shared fc matmul + postprocess structure"""
                pss = [fcps.tile([128, NB], F32, name=f"fcpsum{m}")
                       for m in range(len(mwidths))]
                for k in range(nk):
                    wt = fcw.tile([128, 512], BF16, name="fcwt")
                    nc.sync.dma_start(out=wt[:, 0:wwidth], in_=wdram[k])
                    for m, Mt in enumerate(mwidths):
                        nc.tensor.matmul(
                            pss[m][0:Mt, :],
                            wt[:, m * 128 : m * 128 + Mt],
                            rhs_tile[:, k, :],
                            start=(k == 0), stop=(k == nk - 1),
                        )
                for m, Mt in enumerate(mwidths):
                    sign_out(m, Mt, pss[m])

            # ---- fc1: K=9216 (72 chunks), M=512 slice, N=all images
            ag2_in = dram.tile([512, NB], BF16, name="ag2_in")
            ag2_out = dram.tile([4096, NB], BF16, name="ag2_out", addr_space="Shared")

            def post_fc1(m, Mt, ps):
                sg = fctmp.tile([128, NB], BF16, name="fcsg")
                nc.scalar.activation(sg[0:Mt], ps[0:Mt], AF.Sign, bias=ntau(10 + m, Mt))
                nc.sync.dma_start(out=ag2_in[m * 128 : m * 128 + Mt, :], in_=sg[0:Mt])

            fclayer(72, "f1", d["fw1s"], 512, fc1rhs, [128, 128, 128, 128], post_fc1)
            nc.gpsimd.collective_compute(
                "AllGather", mybir.AluOpType.bypass, replica_groups=rg,
                ins=[ag2_in.opt()], outs=[ag2_out.opt()],
            )

            # ---- fc2
            fc2rhs = fcrhs.tile([128, 32, NB], BF16, name="fc2rhs")
            for k in range(32):
                nc.sync.dma_start(out=fc2rhs[:, k, :], in_=ag2_out[k * 128 : (k + 1) * 128, :])
            ag3_in = dram.tile([512, NB], BF16, name="ag3_in")
            ag3_out = dram.tile([4096, NB], BF16, name="ag3_out", addr_space="Shared")

            def post_fc2(m, Mt, ps):
                sg = fctmp.tile([128, NB], BF16, name="fcsg2")
                nc.scalar.activation(sg[0:Mt], ps[0:Mt], AF.Sign, bias=ntau(14 + m, Mt))
                nc.sync.dma_start(out=ag3_in[m * 128 : m * 128 + Mt, :], in_=sg[0:Mt])

            fclayer(32, "f2", d["fw2s"], 512, fc2rhs, [128, 128, 128, 128], post_fc2)
            nc.gpsimd.collective_compute(
                "AllGather", mybir.AluOpType.bypass, replica_groups=rg,
                ins=[ag3_in.opt()], outs=[ag3_out.opt()],
            )

            # ---- fc3 + bn8 affine
            fc3rhs = fcrhs.tile([128, 32, NB], BF16, name="fc3rhs")
            for k in range(32):
                nc.sync.dma_start(out=fc3rhs[:, k, :], in_=ag3_out[k * 128 : (k + 1) * 128, :])
            ag4_in = dram.tile([125, NB], F32, name="ag4_in")
            ag4_out = dram.tile([1000, NB], F32, name="ag4_out", addr_space="Shared")

            def post_fc3(m, Mt, ps):
                lg = fctmp.tile([128, NB], F32, name="fclg")
                nc.vector.tensor_scalar(
                    lg[0:Mt], ps[0:Mt],
                    taus_sb[0:Mt, 18:19], taus_sb[0:Mt, 19:20],
                    op0=mybir.AluOpType.mult, op1=mybir.AluOpType.add,
                )
                nc.sync.dma_start(out=ag4_in[:, :], in_=lg[0:Mt])

            fclayer(32, "f3", d["fw3s"], 125, fc3rhs, [125], post_fc3)
            nc.gpsimd.collective_compute(
                "AllGather", mybir.AluOpType.bypass, replica_groups=rg,
                ins=[ag4_in.opt()], outs=[ag4_out.opt()],
            )

            # ---- transpose logits to [images, 1000] and log_softmax
            logt = fcrhs.tile([128, 1000], F32, name="logt")
            for t in range(8):
                lin = fctmp.tile([128, NB], F32, name="lin")
                nc.sync.dma_start(out=lin[0:125], in_=ag4_out[t * 125 : (t + 1) * 125, :])
                ltp = tps.tile([128, 128], F32, name="ltp")
                nc.tensor.transpose(ltp[0:NB, 0:125], lin[0:125, 0:NB], idf_sb[0:125, 0:125])
                nc.scalar.activation(
                    logt[0:NB, t * 125 : (t + 1) * 125], ltp[0:NB, 0:125], AF.Copy
                )
            mx = fctmp.tile([128, 1], F32, name="mx", bufs=1)
            nmx = fctmp.tile([128, 1], F32, name="nmx", bufs=1)
            sh = fctmp.tile([128, 1000], F32, name="sh", bufs=1)
            ex = fctmp.tile([128, 1000], F32, name="ex", bufs=1)
            sm = fctmp.tile([128, 1], F32, name="sm", bufs=1)
            lns = fctmp.tile([128, 1], F32, name="lns", bufs=1)
            res = fctmp.tile([128, 1000], F32, name="res", bufs=1)
            nc.vector.tensor_reduce(mx[0:NB], logt[0:NB], axis=mybir.AxisListType.X,
                                    op=mybir.AluOpType.max)
            nc.vector.tensor_scalar_mul(nmx[0:NB], mx[0:NB], -1.0)
            nc.vector.tensor_scalar(sh[0:NB], logt[0:NB], nmx[0:NB], -85.0,
                                    op0=mybir.AluOpType.add, op1=mybir.AluOpType.max)
            nc.scalar.activation(ex[0:NB], sh[0:NB], AF.Exp)
            nc.vector.tensor_reduce(sm[0:NB], ex[0:NB], axis=mybir.AxisListType.X,
                                    op=mybir.AluOpType.add)
            nc.scalar.activation(lns[0:NB], sm[0:NB], AF.Ln)
            nc.vector.tensor_scalar(res[0:NB], logt[0:NB], nmx[0:NB], lns[0:NB],
                                    op0=mybir.AluOpType.add, op1=mybir.AluOpType.subtract)
            nc.sync.dma_start(out=out[:, :], in_=res[0:NB])


# ----------------------------------------------------------------------------
# Host-side preparation
# ----------------------------------------------------------------------------
def _sgn(a):
    return np.where(a >= 0, np.float32(1.0), np.float32(-1.0))


def prep_inputs(inputs, B=16):
    """Full-batch inputs -> per-core in_maps."""
    bf = ml_dtypes.bfloat16
    x = np.asarray(inputs["x"], np.float32)
    NB = x.shape[0]
    assert NB == NCORES * B

    xp = np.zeros((NB, 3, 228, 228), np.float32)
    xp[:, :, 2:226, 2:226] = x
    xph = (
        xp.reshape(NB, 3, 57, 4, 57, 4)
        .transpose(0, 1, 3, 5, 2, 4)
        .reshape(NB, 48, 57, 57)
    )

    wb1 = _sgn(np.asarray(inputs["cw1"], np.float32))  # [64,3,11,11]
    w1p = np.zeros((48, 3, 3, 64), np.float32)
    for ci in range(3):
        for kh in range(11):
            ah, rh = divmod(kh, 4)
            for kw in range(11):
                aw, rw = divmod(kw, 4)
                w1p[ci * 16 + rh * 4 + rw, ah, aw, :] = wb1[:, ci, kh, kw]
    w1f = np.concatenate([w1p[:, :, 0, :], w1p[:, :, 1, :]], axis=0)  # [96,3,64]
    w1h = np.ascontiguousarray(w1p[:, :, 2, :])  # [48,3,64]

    wb2 = _sgn(np.asarray(inputs["cw2"], np.float32))  # [192,64,5,5]
    w2f = np.zeros((128, 10, 192), np.float32)
    w2h = np.zeros((64, 5, 192), np.float32)
    for kh in range(5):
        for p_ in range(2):
            kw = 2 * p_
            w2f[0:64, kh * 2 + p_, :] = wb2[:, :, kh, kw].T
            w2f[64:128, kh * 2 + p_, :] = wb2[:, :, kh, kw + 1].T
        w2h[:, kh, :] = wb2[:, :, kh, 4].T

    wb3 = _sgn(np.asarray(inputs["cw3"], np.float32))  # [384,192,3,3]
    w3a = np.zeros((128, 9, 384), np.float32)
    w3bf = np.zeros((128, 3, 384), np.float32)
    w3bh = np.zeros((64, 3, 384), np.float32)
    for kh in range(3):
        for kw in range(3):
            w3a[:, kh * 3 + kw, :] = wb3[:, 0:128, kh, kw].T
        w3bf[0:64, kh, :] = wb3[:, 128:192, kh, 0].T
        w3bf[64:128, kh, :] = wb3[:, 128:192, kh, 1].T
        w3bh[:, kh, :] = wb3[:, 128:192, kh, 2].T

    wb4 = _sgn(np.asarray(inputs["cw4"], np.float32))  # [256,384,3,3]
    w4 = np.zeros((128, 27, 256), np.float32)
    for b in range(3):
        for kh in range(3):
            for kw in range(3):
                w4[:, b * 9 + kh * 3 + kw, :] = wb4[:, b * 128 : (b + 1) * 128, kh, kw].T
    wb5 = _sgn(np.asarray(inputs["cw5"], np.float32))  # [256,256,3,3]
    w5 = np.zeros((128, 18, 256), np.float32)
    for b in range(2):
        for kh in range(3):
            for kw in range(3):
                w5[:, b * 9 + kh * 3 + kw, :] = wb5[:, b * 128 : (b + 1) * 128, kh, kw].T

    FW1 = _sgn(np.asarray(inputs["fw1"], np.float32))  # [4096, 9216]
    FW2 = _sgn(np.asarray(inputs["fw2"], np.float32))  # [4096, 4096]
    FW3 = _sgn(np.asarray(inputs["fw3"], np.float32))  # [1000, 4096]

    def tau(g, be, m, v, bias):
        g = np.asarray(g, np.float64)
        assert (g > 0).all(), "bn gamma must be positive for the sign fold"
        t = (np.asarray(m, np.float64)
             - np.asarray(be, np.float64) * np.sqrt(np.asarray(v, np.float64) + EPS) / g
             - np.asarray(bias, np.float64))
        return t.astype(np.float32)

    i = inputs
    t1 = tau(i["g1"], i["be1"], i["m1"], i["v1"], i["cb1"])
    t2 = tau(i["g2"], i["be2"], i["m2"], i["v2"], i["cb2"])
    t3 = tau(i["g3"], i["be3"], i["m3"], i["v3"], i["cb3"])
    t4 = tau(i["g4"], i["be4"], i["m4"], i["v4"], i["cb4"])
    t5 = tau(i["g5"], i["be5"], i["m5"], i["v5"], i["cb5"])
    t6 = tau(i["g6"], i["be6"], i["m6"], i["v6"], i["fb1"])
    t7 = tau(i["g7"], i["be7"], i["m7"], i["v7"], i["fb2"])
    g8 = np.asarray(i["g8"], np.float64)
    s8 = np.sqrt(np.asarray(i["v8"], np.float64) + EPS)
    scale8 = (g8 / s8).astype(np.float32)
    bias8 = ((np.asarray(i["fb3"], np.float64) - np.asarray(i["m8"], np.float64)) * g8 / s8
             + np.asarray(i["be8"], np.float64)).astype(np.float32)

    idbf = np.eye(128, dtype=bf)
    idf = np.eye(128, dtype=np.float32)

    shared = {
        "w1f": w1f, "w1h": w1h,
        "w2f": w2f.astype(bf), "w2h": w2h.astype(bf),
        "w3a": w3a.astype(bf), "w3bf": w3bf.astype(bf), "w3bh": w3bh.astype(bf),
        "w4": w4.astype(bf), "w5": w5.astype(bf),
        "idbf": idbf, "idf": idf,
    }

    in_maps = []
    for r in range(NCORES):
        taus_t = np.zeros((128, 20), np.float32)
        taus_t[:, 0] = -np.concatenate([t1, t1])
        taus_t[:, 1] = -t2[0:128]
        taus_t[0:64, 2] = -t2[128:192]
        for m in range(3):
            taus_t[:, 3 + m] = -t3[m * 128 : (m + 1) * 128]
        for m in range(2):
            taus_t[:, 6 + m] = -t4[m * 128 : (m + 1) * 128]
            taus_t[:, 8 + m] = -t5[m * 128 : (m + 1) * 128]
        for m in range(4):
            taus_t[:, 10 + m] = -t6[r * 512 + m * 128 : r * 512 + (m + 1) * 128]
            taus_t[:, 14 + m] = -t7[r * 512 + m * 128 : r * 512 + (m + 1) * 128]
        taus_t[0:125, 18] = scale8[r * 125 : (r + 1) * 125]
        taus_t[0:125, 19] = bias8[r * 125 : (r + 1) * 125]

        fw1s = np.ascontiguousarray(
            FW1[r * 512 : (r + 1) * 512, :].T.reshape(72, 128, 512)
        ).astype(bf)
        fw2s = np.ascontiguousarray(
            FW2[r * 512 : (r + 1) * 512, :].T.reshape(32, 128, 512)
        ).astype(bf)
        fw3s = np.ascontiguousarray(
            FW3[r * 125 : (r + 1) * 125, :].T.reshape(32, 128, 125)
        ).astype(bf)

        im = dict(shared)
        im["xph"] = np.ascontiguousarray(
            xph[r * B : (r + 1) * B].transpose(1, 0, 2, 3)
        )
        im["taus"] = taus_t
        im["fw1s"] = fw1s
        im["fw2s"] = fw2s
        im["fw3s"] = fw3s
        in_maps.append(im)
    return in_maps


_NC_CACHE = {}


def run(inputs, B=16, trace=False, **kw):
    if B not in _NC_CACHE:
        _NC_CACHE[B] = build_nc(B)
    nc = _NC_CACHE[B]
    in_maps = prep_inputs(inputs, B)
    res = run_bass_kernel_spmd(
        nc, in_maps, core_ids=list(range(NCORES)), trace=trace, **kw
    )
    return res.results[0]["out"].astype(np.float32), res


def kernel(**inputs) -> np.ndarray:
    out, _ = run(inputs, B=16)
    return out
